# revision 8
# baseline (speedup 1.0000x reference)
import sys

sys.path.insert(0, "/opt/trn_rl_repo")

import numpy as np
import ml_dtypes

from concourse import bass, bacc, tile, mybir
from concourse.bass_utils import run_bass_kernel_spmd

B, S, N, D = 4, 96, 512, 8
H = 64
OUT = 24
NT = N // 128  # 4 node tiles of 128 partitions
F = H + D     # 72 features in v = [h | x]
FB = F + 1    # +1 ones row for bias

BF16 = mybir.dt.bfloat16
FP32 = mybir.dt.float32

_CACHE = {}


def _build_nc():
    nc = bacc.Bacc(None)
    adjT_d = nc.dram_tensor("adjT", [S, 128, NT, N], BF16, kind="ExternalInput")
    xT_d = nc.dram_tensor("xT", [128, S, NT, D], BF16, kind="ExternalInput")
    wb_d = nc.dram_tensor("wb", [FB, 4 * H], BF16, kind="ExternalInput")
    h0_d = nc.dram_tensor("h0T", [128, NT, H], BF16, kind="ExternalInput")
    c0_d = nc.dram_tensor("c0T", [128, NT, H], FP32, kind="ExternalInput")
    hout_d = nc.dram_tensor("hout", [128, NT, H], FP32, kind="ExternalOutput")

    with tile.TileContext(nc) as tc:
        with (
            tc.tile_pool(name="persist", bufs=1) as persist,
            tc.tile_pool(name="adj", bufs=3) as adjp,
            tc.tile_pool(name="scratch", bufs=3) as scratch,
            tc.tile_pool(name="ps_av", bufs=2, space="PSUM") as ps_av,
            tc.tile_pool(name="ps_g", bufs=2, space="PSUM") as ps_g,
        ):
            X = persist.tile([128, S, NT, D], BF16)   # all timesteps of x
            V = persist.tile([128, NT, F], BF16)      # [h | x] per node tile
            C = persist.tile([128, NT, H], FP32)      # cell state
            WB = persist.tile([FB, 4 * H], BF16)      # [Wh; Wx; b], g-cols x2
            AVT = persist.tile([FB, N], BF16)         # Av^T + ones row
            HF = persist.tile([128, NT, H], FP32)     # final h, fp32
            H0 = persist.tile([128, NT, H], BF16)
            WMOV = persist.tile([128, 512], BF16)     # PE warmup fodder

            nc.sync.dma_start(WB[:], wb_d[:])
            nc.sync.dma_start(H0[:], h0_d[:])
            nc.sync.dma_start(C[:], c0_d[:])
            nc.gpsimd.dma_start(X[:], xT_d[:])
            nc.vector.memset(WMOV[:], 0.0)
            # PE warm-up: ~4us of dense matmul keeps HAM at K=8/8 (2.4 GHz)
            # for the whole kernel; overlaps the initial DMAs.
            WU = ps_av.tile([FB, 256], FP32, name="WU", tag="AvT0")
            for _ in range(40):
                nc.tensor.matmul(WU[0:F, :], WMOV[:, 0:F], WMOV[:, 0:256],
                                 start=True, stop=True)
            # all V producers stay on DVE so matmul LDW needs a single wait
            nc.vector.tensor_copy(V[:, :, 0:H], H0[:])
            # ones row (72) for bias; partition offset must be mult of 32, so
            # memset 64:73 once — rows 64:72 are rewritten with data each step.
            nc.vector.memset(AVT[64:FB, :], 1.0)

            for s in range(S):
                AT = adjp.tile([128, NT, N], BF16, name="AT", tag="AT")
                nc.sync.dma_start(AT[:], adjT_d[s])

                # x_s into V x slots (SBUF -> SBUF)
                nc.vector.tensor_copy(V[:, :, H : H + D], X[:, s, :, :])

                # mm1 in two n-halves: half-0 cast overlaps half-1 MMs.
                AvT0 = ps_av.tile([FB, 256], FP32, name="AvT0", tag="AvT0")
                AvT1 = ps_av.tile([FB, 256], FP32, name="AvT1", tag="AvT1")
                # G per half: [128, nt(2), 4H] = one PSUM bank
                GA = ps_g.tile([128, 2, 4 * H], FP32, name="GA", tag="GA")
                GB = ps_g.tile([128, 2, 4 * H], FP32, name="GB", tag="GB")
                # gates, gate-major: [128, gate(i,f,o,t'), nt, 64] so each
                # gate slice is contiguous per partition (DVE 2x mode)
                SG = scratch.tile([128, 4, NT, H], BF16, name="SG", tag="SG")
                U = scratch.tile([128, NT, H], BF16, name="U", tag="U")
                FC = scratch.tile([128, NT, H], FP32, name="FC", tag="FC")
                TC = scratch.tile([128, NT, H], BF16, name="TC", tag="TC")

                # HAM keeper: one dummy MM per step prevents a fully-idle
                # 3.4us PE window (re-throttle); real mm2 overwrites GA.
                nc.tensor.matmul(GA[:, 0, 0:128], WMOV[:, 0:128],
                                 WMOV[:, 0:128], start=True, stop=True)
                for mt in range(NT):
                    nc.tensor.matmul(
                        AvT0[0:F, :],
                        V[:, mt, :],
                        AT[:, mt, 0:256],
                        start=(mt == 0),
                        stop=(mt == NT - 1),
                    )
                nc.vector.tensor_copy(AVT[0:F, 0:256], AvT0[0:F, :])
                for mt in range(NT):
                    nc.tensor.matmul(
                        AvT1[0:F, :],
                        V[:, mt, :],
                        AT[:, mt, 256:512],
                        start=(mt == 0),
                        stop=(mt == NT - 1),
                    )
                nc.vector.tensor_copy(AVT[0:F, 256:512], AvT1[0:F, :])

                for nt in range(2):
                    nc.tensor.matmul(
                        GA[:, nt, :],
                        AVT[:, nt * 128 : (nt + 1) * 128],
                        WB[:],
                        start=True,
                        stop=True,
                    )
                # A-half: per-nt chains (critical path into next mm1 mt0/1)
                for nt in range(2):
                    nc.scalar.activation(
                        SG[:, 0:3, nt, :],
                        GA[:, nt, 0 : 3 * H],
                        mybir.ActivationFunctionType.Sigmoid,
                    )
                    nc.scalar.activation(
                        SG[:, 3, nt, :],
                        GA[:, nt, 3 * H : 4 * H],
                        mybir.ActivationFunctionType.Tanh,
                    )
                for nt in range(2, NT):
                    nc.tensor.matmul(
                        GB[:, nt - 2, :],
                        AVT[:, nt * 128 : (nt + 1) * 128],
                        WB[:],
                        start=True,
                        stop=True,
                    )
                for nt in range(2):
                    nc.vector.tensor_mul(
                        U[:, nt, :], SG[:, 0, nt, :], SG[:, 3, nt, :]
                    )
                    nc.gpsimd.tensor_tensor(
                        FC[:, nt, :], SG[:, 1, nt, :], C[:, nt, :],
                        mybir.AluOpType.mult,
                    )
                    nc.vector.scalar_tensor_tensor(
                        C[:, nt, :], FC[:, nt, :], 1.0, U[:, nt, :],
                        mybir.AluOpType.bypass, mybir.AluOpType.add,
                    )
                    nc.scalar.activation(
                        TC[:, nt, :], C[:, nt, :],
                        mybir.ActivationFunctionType.Tanh,
                    )
                    if s == S - 1:
                        nc.vector.scalar_tensor_tensor(
                            HF[:, nt, :], SG[:, 2, nt, :], 1.0, TC[:, nt, :],
                            mybir.AluOpType.bypass, mybir.AluOpType.mult,
                        )
                    elif nt == 0:
                        nc.vector.scalar_tensor_tensor(
                            V[:, nt, 0:H], SG[:, 2, nt, :], 1.0, TC[:, nt, :],
                            mybir.AluOpType.bypass, mybir.AluOpType.mult,
                        )
                    else:
                        nc.gpsimd.tensor_tensor(
                            V[:, nt, 0:H], SG[:, 2, nt, :], TC[:, nt, :],
                            mybir.AluOpType.mult,
                        )
                # B-half: per-half (fewer op inits; has slack behind A)
                nc.scalar.activation(
                    SG[:, 0:3, 2:NT, :].transpose((0, 2, 1, 3)),
                    GB[:, :, 0 : 3 * H],
                    mybir.ActivationFunctionType.Sigmoid,
                )
                nc.scalar.activation(
                    SG[:, 3, 2:NT, :],
                    GB[:, :, 3 * H : 4 * H],
                    mybir.ActivationFunctionType.Tanh,
                )
                nc.vector.tensor_mul(
                    U[:, 2:NT, :], SG[:, 0, 2:NT, :], SG[:, 3, 2:NT, :]
                )
                nc.gpsimd.tensor_tensor(
                    FC[:, 2:NT, :], SG[:, 1, 2:NT, :], C[:, 2:NT, :],
                    mybir.AluOpType.mult,
                )
                nc.vector.scalar_tensor_tensor(
                    C[:, 2:NT, :], FC[:, 2:NT, :], 1.0, U[:, 2:NT, :],
                    mybir.AluOpType.bypass, mybir.AluOpType.add,
                )
                nc.scalar.activation(
                    TC[:, 2:NT, :], C[:, 2:NT, :],
                    mybir.ActivationFunctionType.Tanh,
                )
                if s == S - 1:
                    nc.vector.scalar_tensor_tensor(
                        HF[:, 2:NT, :], SG[:, 2, 2:NT, :], 1.0, TC[:, 2:NT, :],
                        mybir.AluOpType.bypass, mybir.AluOpType.mult,
                    )
                else:
                    nc.vector.scalar_tensor_tensor(
                        V[:, 2, 0:H], SG[:, 2, 2, :], 1.0, TC[:, 2, :],
                        mybir.AluOpType.bypass, mybir.AluOpType.mult,
                    )
                    nc.gpsimd.tensor_tensor(
                        V[:, 3, 0:H], SG[:, 2, 3, :], TC[:, 3, :],
                        mybir.AluOpType.mult,
                    )

            nc.sync.dma_start(hout_d[:], HF[:])

    nc.finalize()  # Bacc.finalize runs the multi-wait-splitting passes
    return nc


def _prep_core_inputs(b, x, adj, h0, c0, Wh, Wx, b_gates):
    bf16 = ml_dtypes.bfloat16
    # adjT[s, p, mt, n] = adj[b, s, n, mt*128+p]  (= A_s^T row m, col n)
    a = adj[b].transpose(0, 2, 1).reshape(S, NT, 128, N).transpose((0, 2, 1, 3))
    adjT = np.ascontiguousarray(a, dtype=bf16)
    # xT[p, s, mt, d] = x[b, s, mt*128+p, d]
    xb = x[b].reshape(S, NT, 128, D).transpose(2, 0, 1, 3)
    xT = np.ascontiguousarray(xb, dtype=bf16)
    # h0T/c0T[p, nt, j] = state[b, nt*128+p, j]
    h0b = h0[b].reshape(NT, 128, H).transpose(1, 0, 2)
    c0b = c0[b].reshape(NT, 128, H).transpose(1, 0, 2)
    h0T = np.ascontiguousarray(h0b, dtype=bf16)
    c0T = np.ascontiguousarray(c0b, dtype=np.float32)
    wb = np.concatenate([Wh, Wx, b_gates[None, :]], axis=0).astype(np.float32)
    wb16 = wb.astype(bf16)
    return {"adjT": adjT, "xT": xT, "wb": wb16, "h0T": h0T, "c0T": c0T}


def make_profile_args(inputs):
    """Return (nc, in_maps, core_ids) for test.py's --profile path."""
    x = np.asarray(inputs["x"], np.float32)
    adj = np.asarray(inputs["adj"], np.float32)
    h0 = np.asarray(inputs["initial_hidden_state"], np.float32)
    c0 = np.asarray(inputs["initial_cell_state"], np.float32)
    Wx_ = np.asarray(inputs["Wx"], np.float32)
    Wh_ = np.asarray(inputs["Wh"], np.float32)
    bg = np.asarray(inputs["b_gates"], np.float32)
    if "nc" not in _CACHE:
        _CACHE["nc"] = _build_nc()
    nc = _CACHE["nc"]
    in_maps = [_prep_core_inputs(b, x, adj, h0, c0, Wh_, Wx_, bg) for b in range(B)]
    return nc, in_maps, list(range(B))


def kernel(x, adj, initial_hidden_state, initial_cell_state, Wx, Wh, b_gates,
           W1, b1, W2, b2):
    x = np.asarray(x, dtype=np.float32)
    adj = np.asarray(adj, dtype=np.float32)
    h0 = np.asarray(initial_hidden_state, dtype=np.float32)
    c0 = np.asarray(initial_cell_state, dtype=np.float32)
    Wx_ = np.asarray(Wx, dtype=np.float32)
    Wh_ = np.asarray(Wh, dtype=np.float32)
    bg = np.asarray(b_gates, dtype=np.float32)

    if "nc" not in _CACHE:
        _CACHE["nc"] = _build_nc()
    nc = _CACHE["nc"]

    core_ids = list(range(B))
    in_maps = [_prep_core_inputs(b, x, adj, h0, c0, Wh_, Wx_, bg) for b in range(B)]
    res = run_bass_kernel_spmd(nc, in_maps, core_ids)

    h_final = np.zeros((B, N, H), dtype=np.float32)
    for i in range(B):
        hout = np.asarray(res.results[i]["hout"], dtype=np.float32)  # [128, NT, H]
        h_final[i] = hout.transpose(1, 0, 2).reshape(N, H)

    read_out = h_final[:, 0, :]  # (B, H) -- TARGET_NODE = 0
    pre = read_out @ np.asarray(W1, dtype=np.float32) + np.asarray(b1, dtype=np.float32)
    out = np.maximum(pre, 0.0) @ np.asarray(W2, dtype=np.float32) + np.asarray(
        b2, dtype=np.float32
    )
    return out.astype(np.float32)


# revision 9
# speedup vs baseline: 1.2770x; 1.2770x over previous
import sys

sys.path.insert(0, "/opt/trn_rl_repo")

import numpy as np
import ml_dtypes

from concourse import bass, bacc, tile, mybir
from concourse.bass_utils import run_bass_kernel_spmd

B, S, N, D = 4, 96, 512, 8
H = 64
OUT = 24
NT = N // 128  # 4 node tiles of 128 partitions
F = H + D     # 72 features in v = [h | x]
FB = F + 1    # +1 ones row for bias

BF16 = mybir.dt.bfloat16
FP32 = mybir.dt.float32

_CACHE = {}


def _build_nc():
    nc = bacc.Bacc(None)
    adjT_d = nc.dram_tensor("adjT", [S, 128, NT, N], BF16, kind="ExternalInput")
    xT_d = nc.dram_tensor("xT", [128, S, NT, D], BF16, kind="ExternalInput")
    wb_d = nc.dram_tensor("wb", [FB, 4 * H], BF16, kind="ExternalInput")
    h0_d = nc.dram_tensor("h0T", [128, NT, H], BF16, kind="ExternalInput")
    c0_d = nc.dram_tensor("c0T", [128, NT, H], FP32, kind="ExternalInput")
    hout_d = nc.dram_tensor("hout", [128, NT, H], FP32, kind="ExternalOutput")

    with tile.TileContext(nc) as tc:
        with (
            tc.tile_pool(name="persist", bufs=1) as persist,
            tc.tile_pool(name="adj", bufs=3) as adjp,
            tc.tile_pool(name="scratch", bufs=3) as scratch,
            tc.tile_pool(name="ps_av", bufs=2, space="PSUM") as ps_av,
            tc.tile_pool(name="ps_g", bufs=2, space="PSUM") as ps_g,
        ):
            X = persist.tile([128, S, NT, D], BF16)   # all timesteps of x
            V = persist.tile([128, NT, F], BF16)      # [h | x] per node tile
            C = persist.tile([128, NT, H], FP32)      # cell state
            WB = persist.tile([FB, 4 * H], BF16)      # [Wh; Wx; b], g-cols x2
            AVT = persist.tile([FB, N], BF16)         # Av^T + ones row
            HF = persist.tile([128, NT, H], FP32)     # final h, fp32
            H0 = persist.tile([128, NT, H], BF16)

            nc.sync.dma_start(WB[:], wb_d[:])
            nc.sync.dma_start(H0[:], h0_d[:])
            nc.sync.dma_start(C[:], c0_d[:])
            nc.gpsimd.dma_start(X[:], xT_d[:])
            # all V producers stay on DVE so matmul LDW needs a single wait
            nc.vector.tensor_copy(V[:, :, 0:H], H0[:])
            # ones row (72) for bias; partition offset must be mult of 32, so
            # memset 64:73 once — rows 64:72 are rewritten with data each step.
            nc.vector.memset(AVT[64:FB, :], 1.0)

            for s in range(S):
                AT = adjp.tile([128, NT, N], BF16, name="AT", tag="AT")
                nc.sync.dma_start(AT[:], adjT_d[s])

                # x_s into V x slots (SBUF -> SBUF)
                nc.vector.tensor_copy(V[:, :, H : H + D], X[:, s, :, :])

                # mm1 in two n-halves: half-0 cast overlaps half-1 MMs.
                AvT0 = ps_av.tile([FB, 256], FP32, name="AvT0", tag="AvT0")
                AvT1 = ps_av.tile([FB, 256], FP32, name="AvT1", tag="AvT1")
                # G per half: [128, nt(2), 4H] = one PSUM bank
                GA = ps_g.tile([128, 2, 4 * H], FP32, name="GA", tag="GA")
                GB = ps_g.tile([128, 2, 4 * H], FP32, name="GB", tag="GB")
                # gates, gate-major: [128, gate(i,f,o,t'), nt, 64] so each
                # gate slice is contiguous per partition (DVE 2x mode)
                SG = scratch.tile([128, 4, NT, H], BF16, name="SG", tag="SG")
                U = scratch.tile([128, NT, H], BF16, name="U", tag="U")
                FC = scratch.tile([128, NT, H], FP32, name="FC", tag="FC")
                TC = scratch.tile([128, NT, H], BF16, name="TC", tag="TC")

                for mt in range(NT):
                    nc.tensor.matmul(
                        AvT0[0:F, :],
                        V[:, mt, :],
                        AT[:, mt, 0:256],
                        start=(mt == 0),
                        stop=(mt == NT - 1),
                    )
                nc.scalar.activation(
                    AVT[0:F, 0:256], AvT0[0:F, :],
                    mybir.ActivationFunctionType.Copy,
                )
                for mt in range(NT):
                    nc.tensor.matmul(
                        AvT1[0:F, :],
                        V[:, mt, :],
                        AT[:, mt, 256:512],
                        start=(mt == 0),
                        stop=(mt == NT - 1),
                    )
                nc.scalar.activation(
                    AVT[0:F, 256:384], AvT1[0:F, 0:128],
                    mybir.ActivationFunctionType.Copy,
                )
                nc.vector.tensor_copy(AVT[0:F, 384:512], AvT1[0:F, 128:256])

                for nt in range(2):
                    nc.tensor.matmul(
                        GA[:, nt, :],
                        AVT[:, nt * 128 : (nt + 1) * 128],
                        WB[:],
                        start=True,
                        stop=True,
                    )
                # A-half: per-nt chains (critical path into next mm1 mt0/1)
                for nt in range(2):
                    nc.scalar.activation(
                        SG[:, 0:3, nt, :],
                        GA[:, nt, 0 : 3 * H],
                        mybir.ActivationFunctionType.Sigmoid,
                    )
                    nc.scalar.activation(
                        SG[:, 3, nt, :],
                        GA[:, nt, 3 * H : 4 * H],
                        mybir.ActivationFunctionType.Tanh,
                    )
                for nt in range(2, NT):
                    nc.tensor.matmul(
                        GB[:, nt - 2, :],
                        AVT[:, nt * 128 : (nt + 1) * 128],
                        WB[:],
                        start=True,
                        stop=True,
                    )
                for nt in range(2):
                    nc.vector.tensor_mul(
                        U[:, nt, :], SG[:, 0, nt, :], SG[:, 3, nt, :]
                    )
                    nc.gpsimd.tensor_tensor(
                        FC[:, nt, :], SG[:, 1, nt, :], C[:, nt, :],
                        mybir.AluOpType.mult,
                    )
                    nc.vector.scalar_tensor_tensor(
                        C[:, nt, :], FC[:, nt, :], 1.0, U[:, nt, :],
                        mybir.AluOpType.bypass, mybir.AluOpType.add,
                    )
                    nc.scalar.activation(
                        TC[:, nt, :], C[:, nt, :],
                        mybir.ActivationFunctionType.Tanh,
                    )
                    if s == S - 1:
                        nc.vector.scalar_tensor_tensor(
                            HF[:, nt, :], SG[:, 2, nt, :], 1.0, TC[:, nt, :],
                            mybir.AluOpType.bypass, mybir.AluOpType.mult,
                        )
                    elif nt == 0:
                        nc.vector.scalar_tensor_tensor(
                            V[:, nt, 0:H], SG[:, 2, nt, :], 1.0, TC[:, nt, :],
                            mybir.AluOpType.bypass, mybir.AluOpType.mult,
                        )
                    else:
                        nc.gpsimd.tensor_tensor(
                            V[:, nt, 0:H], SG[:, 2, nt, :], TC[:, nt, :],
                            mybir.AluOpType.mult,
                        )
                # B-half: per-half (fewer op inits; has slack behind A)
                nc.scalar.activation(
                    SG[:, 0:3, 2:NT, :].transpose((0, 2, 1, 3)),
                    GB[:, :, 0 : 3 * H],
                    mybir.ActivationFunctionType.Sigmoid,
                )
                nc.scalar.activation(
                    SG[:, 3, 2:NT, :],
                    GB[:, :, 3 * H : 4 * H],
                    mybir.ActivationFunctionType.Tanh,
                )
                nc.vector.tensor_mul(
                    U[:, 2:NT, :], SG[:, 0, 2:NT, :], SG[:, 3, 2:NT, :]
                )
                nc.gpsimd.tensor_tensor(
                    FC[:, 2:NT, :], SG[:, 1, 2:NT, :], C[:, 2:NT, :],
                    mybir.AluOpType.mult,
                )
                nc.vector.scalar_tensor_tensor(
                    C[:, 2:NT, :], FC[:, 2:NT, :], 1.0, U[:, 2:NT, :],
                    mybir.AluOpType.bypass, mybir.AluOpType.add,
                )
                nc.scalar.activation(
                    TC[:, 2:NT, :], C[:, 2:NT, :],
                    mybir.ActivationFunctionType.Tanh,
                )
                if s == S - 1:
                    nc.vector.scalar_tensor_tensor(
                        HF[:, 2:NT, :], SG[:, 2, 2:NT, :], 1.0, TC[:, 2:NT, :],
                        mybir.AluOpType.bypass, mybir.AluOpType.mult,
                    )
                else:
                    nc.vector.scalar_tensor_tensor(
                        V[:, 2, 0:H], SG[:, 2, 2, :], 1.0, TC[:, 2, :],
                        mybir.AluOpType.bypass, mybir.AluOpType.mult,
                    )
                    nc.gpsimd.tensor_tensor(
                        V[:, 3, 0:H], SG[:, 2, 3, :], TC[:, 3, :],
                        mybir.AluOpType.mult,
                    )

            nc.sync.dma_start(hout_d[:], HF[:])

    nc.finalize()  # Bacc.finalize runs the multi-wait-splitting passes
    return nc


def _prep_core_inputs(b, x, adj, h0, c0, Wh, Wx, b_gates):
    bf16 = ml_dtypes.bfloat16
    # adjT[s, p, mt, n] = adj[b, s, n, mt*128+p]  (= A_s^T row m, col n)
    a = adj[b].transpose(0, 2, 1).reshape(S, NT, 128, N).transpose((0, 2, 1, 3))
    adjT = np.ascontiguousarray(a, dtype=bf16)
    # xT[p, s, mt, d] = x[b, s, mt*128+p, d]
    xb = x[b].reshape(S, NT, 128, D).transpose(2, 0, 1, 3)
    xT = np.ascontiguousarray(xb, dtype=bf16)
    # h0T/c0T[p, nt, j] = state[b, nt*128+p, j]
    h0b = h0[b].reshape(NT, 128, H).transpose(1, 0, 2)
    c0b = c0[b].reshape(NT, 128, H).transpose(1, 0, 2)
    h0T = np.ascontiguousarray(h0b, dtype=bf16)
    c0T = np.ascontiguousarray(c0b, dtype=np.float32)
    wb = np.concatenate([Wh, Wx, b_gates[None, :]], axis=0).astype(np.float32)
    wb16 = wb.astype(bf16)
    return {"adjT": adjT, "xT": xT, "wb": wb16, "h0T": h0T, "c0T": c0T}


def make_profile_args(inputs):
    """Return (nc, in_maps, core_ids) for test.py's --profile path."""
    x = np.asarray(inputs["x"], np.float32)
    adj = np.asarray(inputs["adj"], np.float32)
    h0 = np.asarray(inputs["initial_hidden_state"], np.float32)
    c0 = np.asarray(inputs["initial_cell_state"], np.float32)
    Wx_ = np.asarray(inputs["Wx"], np.float32)
    Wh_ = np.asarray(inputs["Wh"], np.float32)
    bg = np.asarray(inputs["b_gates"], np.float32)
    if "nc" not in _CACHE:
        _CACHE["nc"] = _build_nc()
    nc = _CACHE["nc"]
    in_maps = [_prep_core_inputs(b, x, adj, h0, c0, Wh_, Wx_, bg) for b in range(B)]
    return nc, in_maps, list(range(B))


def kernel(x, adj, initial_hidden_state, initial_cell_state, Wx, Wh, b_gates,
           W1, b1, W2, b2):
    x = np.asarray(x, dtype=np.float32)
    adj = np.asarray(adj, dtype=np.float32)
    h0 = np.asarray(initial_hidden_state, dtype=np.float32)
    c0 = np.asarray(initial_cell_state, dtype=np.float32)
    Wx_ = np.asarray(Wx, dtype=np.float32)
    Wh_ = np.asarray(Wh, dtype=np.float32)
    bg = np.asarray(b_gates, dtype=np.float32)

    if "nc" not in _CACHE:
        _CACHE["nc"] = _build_nc()
    nc = _CACHE["nc"]

    core_ids = list(range(B))
    in_maps = [_prep_core_inputs(b, x, adj, h0, c0, Wh_, Wx_, bg) for b in range(B)]
    res = run_bass_kernel_spmd(nc, in_maps, core_ids)

    h_final = np.zeros((B, N, H), dtype=np.float32)
    for i in range(B):
        hout = np.asarray(res.results[i]["hout"], dtype=np.float32)  # [128, NT, H]
        h_final[i] = hout.transpose(1, 0, 2).reshape(N, H)

    read_out = h_final[:, 0, :]  # (B, H) -- TARGET_NODE = 0
    pre = read_out @ np.asarray(W1, dtype=np.float32) + np.asarray(b1, dtype=np.float32)
    out = np.maximum(pre, 0.0) @ np.asarray(W2, dtype=np.float32) + np.asarray(
        b2, dtype=np.float32
    )
    return out.astype(np.float32)


# revision 10
# speedup vs baseline: 1.3322x; 1.0433x over previous
import sys

sys.path.insert(0, "/opt/trn_rl_repo")

import numpy as np
import ml_dtypes

from concourse import bass, bacc, tile, mybir
from concourse.bass_utils import run_bass_kernel_spmd

B, S, N, D = 4, 96, 512, 8
H = 64
OUT = 24
NT = N // 128  # 4 node tiles of 128 partitions
F = H + D     # 72 features in v = [h | x]
FB = F + 1    # +1 ones row for bias

BF16 = mybir.dt.bfloat16
FP32 = mybir.dt.float32

_CACHE = {}


def _build_nc():
    nc = bacc.Bacc(None)
    adjT_d = nc.dram_tensor("adjT", [S, 128, NT, N], BF16, kind="ExternalInput")
    xT_d = nc.dram_tensor("xT", [128, S, NT, D], BF16, kind="ExternalInput")
    wb_d = nc.dram_tensor("wb", [FB, 4 * H], BF16, kind="ExternalInput")
    h0_d = nc.dram_tensor("h0T", [128, NT, H], BF16, kind="ExternalInput")
    c0_d = nc.dram_tensor("c0T", [128, NT, H], FP32, kind="ExternalInput")
    hout_d = nc.dram_tensor("hout", [128, NT, H], FP32, kind="ExternalOutput")

    with tile.TileContext(nc) as tc:
        with (
            tc.tile_pool(name="persist", bufs=1) as persist,
            tc.tile_pool(name="adj", bufs=3) as adjp,
            tc.tile_pool(name="scratch", bufs=3) as scratch,
            tc.tile_pool(name="ps_av", bufs=2, space="PSUM") as ps_av,
            tc.tile_pool(name="ps_g", bufs=4, space="PSUM") as ps_g,
        ):
            X = persist.tile([128, S, NT, D], BF16)   # all timesteps of x
            V = persist.tile([128, NT, F], BF16)      # [h | x] per node tile
            C = persist.tile([128, NT, H], FP32)      # cell state
            WB = persist.tile([FB, 4 * H], BF16)      # [Wh; Wx; b], g-cols x2
            AVT = persist.tile([FB, N], BF16)         # Av^T + ones row
            HF = persist.tile([128, NT, H], FP32)     # final h, fp32
            H0 = persist.tile([128, NT, H], BF16)

            nc.sync.dma_start(WB[:], wb_d[:])
            nc.sync.dma_start(H0[:], h0_d[:])
            nc.sync.dma_start(C[:], c0_d[:])
            nc.gpsimd.dma_start(X[:], xT_d[:])
            # all V producers stay on DVE so matmul LDW needs a single wait
            nc.vector.tensor_copy(V[:, :, 0:H], H0[:])
            # ones row (72) for bias; partition offset must be mult of 32, so
            # memset 64:73 once — rows 64:72 are rewritten with data each step.
            nc.vector.memset(AVT[64:FB, :], 1.0)

            for s in range(S):
                AT = adjp.tile([128, NT, N], BF16, name="AT", tag="AT")
                nc.sync.dma_start(AT[:], adjT_d[s])

                # x_s into V x slots (SBUF -> SBUF)
                nc.vector.tensor_copy(V[:, :, H : H + D], X[:, s, :, :])

                # mm1 in two n-halves: half-0 cast overlaps half-1 MMs.
                AvT0 = ps_av.tile([FB, 256], FP32, name="AvT0", tag="AvT0")
                AvT1 = ps_av.tile([FB, 256], FP32, name="AvT1", tag="AvT1")
                # G per nt: separate PSUM banks so each sigmoid depends
                # only on its own mm2 matmul (PSUM deps are bank-level)
                G = [ps_g.tile([128, 4 * H], FP32, name=f"G{nt}", tag="G")
                     for nt in range(NT)]
                # gates, gate-major: [128, gate(i,f,o,t'), nt, 64] so each
                # gate slice is contiguous per partition (DVE 2x mode)
                SG = scratch.tile([128, 4, NT, H], BF16, name="SG", tag="SG")
                U = scratch.tile([128, NT, H], BF16, name="U", tag="U")
                FC = scratch.tile([128, NT, H], FP32, name="FC", tag="FC")
                TC = scratch.tile([128, NT, H], BF16, name="TC", tag="TC")

                for mt in range(NT):
                    nc.tensor.matmul(
                        AvT0[0:F, :],
                        V[:, mt, :],
                        AT[:, mt, 0:256],
                        start=(mt == 0),
                        stop=(mt == NT - 1),
                    )
                nc.scalar.activation(
                    AVT[0:F, 0:128], AvT0[0:F, 0:128],
                    mybir.ActivationFunctionType.Copy,
                )
                nc.vector.tensor_copy(AVT[0:F, 128:256], AvT0[0:F, 128:256])
                for mt in range(NT):
                    nc.tensor.matmul(
                        AvT1[0:F, :],
                        V[:, mt, :],
                        AT[:, mt, 256:512],
                        start=(mt == 0),
                        stop=(mt == NT - 1),
                    )
                nc.scalar.activation(
                    AVT[0:F, 256:384], AvT1[0:F, 0:128],
                    mybir.ActivationFunctionType.Copy,
                )
                nc.vector.tensor_copy(AVT[0:F, 384:512], AvT1[0:F, 128:256])

                # per-nt: mm2 -> its own PSUM bank -> sigmoid/tanh chain
                for nt in range(NT):
                    nc.tensor.matmul(
                        G[nt][:, :],
                        AVT[:, nt * 128 : (nt + 1) * 128],
                        WB[:],
                        start=True,
                        stop=True,
                    )
                    nc.scalar.activation(
                        SG[:, 0:3, nt, :],
                        G[nt][:, 0 : 3 * H],
                        mybir.ActivationFunctionType.Sigmoid,
                    )
                    nc.scalar.activation(
                        SG[:, 3, nt, :],
                        G[nt][:, 3 * H : 4 * H],
                        mybir.ActivationFunctionType.Tanh,
                    )
                for nt in range(2):
                    nc.vector.tensor_mul(
                        U[:, nt, :], SG[:, 0, nt, :], SG[:, 3, nt, :]
                    )
                    nc.gpsimd.tensor_tensor(
                        FC[:, nt, :], SG[:, 1, nt, :], C[:, nt, :],
                        mybir.AluOpType.mult,
                    )
                    nc.vector.scalar_tensor_tensor(
                        C[:, nt, :], FC[:, nt, :], 1.0, U[:, nt, :],
                        mybir.AluOpType.bypass, mybir.AluOpType.add,
                    )
                    nc.scalar.activation(
                        TC[:, nt, :], C[:, nt, :],
                        mybir.ActivationFunctionType.Tanh,
                    )
                    if s == S - 1:
                        nc.vector.scalar_tensor_tensor(
                            HF[:, nt, :], SG[:, 2, nt, :], 1.0, TC[:, nt, :],
                            mybir.AluOpType.bypass, mybir.AluOpType.mult,
                        )
                    elif nt == 0:
                        nc.vector.scalar_tensor_tensor(
                            V[:, nt, 0:H], SG[:, 2, nt, :], 1.0, TC[:, nt, :],
                            mybir.AluOpType.bypass, mybir.AluOpType.mult,
                        )
                    else:
                        nc.gpsimd.tensor_tensor(
                            V[:, nt, 0:H], SG[:, 2, nt, :], TC[:, nt, :],
                            mybir.AluOpType.mult,
                        )
                # B-half cell update per-half (has slack behind A)
                nc.vector.tensor_mul(
                    U[:, 2:NT, :], SG[:, 0, 2:NT, :], SG[:, 3, 2:NT, :]
                )
                nc.gpsimd.tensor_tensor(
                    FC[:, 2:NT, :], SG[:, 1, 2:NT, :], C[:, 2:NT, :],
                    mybir.AluOpType.mult,
                )
                nc.vector.scalar_tensor_tensor(
                    C[:, 2:NT, :], FC[:, 2:NT, :], 1.0, U[:, 2:NT, :],
                    mybir.AluOpType.bypass, mybir.AluOpType.add,
                )
                nc.scalar.activation(
                    TC[:, 2:NT, :], C[:, 2:NT, :],
                    mybir.ActivationFunctionType.Tanh,
                )
                if s == S - 1:
                    nc.vector.scalar_tensor_tensor(
                        HF[:, 2:NT, :], SG[:, 2, 2:NT, :], 1.0, TC[:, 2:NT, :],
                        mybir.AluOpType.bypass, mybir.AluOpType.mult,
                    )
                else:
                    nc.vector.scalar_tensor_tensor(
                        V[:, 2, 0:H], SG[:, 2, 2, :], 1.0, TC[:, 2, :],
                        mybir.AluOpType.bypass, mybir.AluOpType.mult,
                    )
                    nc.gpsimd.tensor_tensor(
                        V[:, 3, 0:H], SG[:, 2, 3, :], TC[:, 3, :],
                        mybir.AluOpType.mult,
                    )

            nc.sync.dma_start(hout_d[:], HF[:])

    nc.finalize()  # Bacc.finalize runs the multi-wait-splitting passes
    return nc


def _prep_core_inputs(b, x, adj, h0, c0, Wh, Wx, b_gates):
    bf16 = ml_dtypes.bfloat16
    # adjT[s, p, mt, n] = adj[b, s, n, mt*128+p]  (= A_s^T row m, col n)
    a = adj[b].transpose(0, 2, 1).reshape(S, NT, 128, N).transpose((0, 2, 1, 3))
    adjT = np.ascontiguousarray(a, dtype=bf16)
    # xT[p, s, mt, d] = x[b, s, mt*128+p, d]
    xb = x[b].reshape(S, NT, 128, D).transpose(2, 0, 1, 3)
    xT = np.ascontiguousarray(xb, dtype=bf16)
    # h0T/c0T[p, nt, j] = state[b, nt*128+p, j]
    h0b = h0[b].reshape(NT, 128, H).transpose(1, 0, 2)
    c0b = c0[b].reshape(NT, 128, H).transpose(1, 0, 2)
    h0T = np.ascontiguousarray(h0b, dtype=bf16)
    c0T = np.ascontiguousarray(c0b, dtype=np.float32)
    wb = np.concatenate([Wh, Wx, b_gates[None, :]], axis=0).astype(np.float32)
    wb16 = wb.astype(bf16)
    return {"adjT": adjT, "xT": xT, "wb": wb16, "h0T": h0T, "c0T": c0T}


def make_profile_args(inputs):
    """Return (nc, in_maps, core_ids) for test.py's --profile path."""
    x = np.asarray(inputs["x"], np.float32)
    adj = np.asarray(inputs["adj"], np.float32)
    h0 = np.asarray(inputs["initial_hidden_state"], np.float32)
    c0 = np.asarray(inputs["initial_cell_state"], np.float32)
    Wx_ = np.asarray(inputs["Wx"], np.float32)
    Wh_ = np.asarray(inputs["Wh"], np.float32)
    bg = np.asarray(inputs["b_gates"], np.float32)
    if "nc" not in _CACHE:
        _CACHE["nc"] = _build_nc()
    nc = _CACHE["nc"]
    in_maps = [_prep_core_inputs(b, x, adj, h0, c0, Wh_, Wx_, bg) for b in range(B)]
    return nc, in_maps, list(range(B))


def kernel(x, adj, initial_hidden_state, initial_cell_state, Wx, Wh, b_gates,
           W1, b1, W2, b2):
    x = np.asarray(x, dtype=np.float32)
    adj = np.asarray(adj, dtype=np.float32)
    h0 = np.asarray(initial_hidden_state, dtype=np.float32)
    c0 = np.asarray(initial_cell_state, dtype=np.float32)
    Wx_ = np.asarray(Wx, dtype=np.float32)
    Wh_ = np.asarray(Wh, dtype=np.float32)
    bg = np.asarray(b_gates, dtype=np.float32)

    if "nc" not in _CACHE:
        _CACHE["nc"] = _build_nc()
    nc = _CACHE["nc"]

    core_ids = list(range(B))
    in_maps = [_prep_core_inputs(b, x, adj, h0, c0, Wh_, Wx_, bg) for b in range(B)]
    res = run_bass_kernel_spmd(nc, in_maps, core_ids)

    h_final = np.zeros((B, N, H), dtype=np.float32)
    for i in range(B):
        hout = np.asarray(res.results[i]["hout"], dtype=np.float32)  # [128, NT, H]
        h_final[i] = hout.transpose(1, 0, 2).reshape(N, H)

    read_out = h_final[:, 0, :]  # (B, H) -- TARGET_NODE = 0
    pre = read_out @ np.asarray(W1, dtype=np.float32) + np.asarray(b1, dtype=np.float32)
    out = np.maximum(pre, 0.0) @ np.asarray(W2, dtype=np.float32) + np.asarray(
        b2, dtype=np.float32
    )
    return out.astype(np.float32)
